# revision 2
# baseline (speedup 1.0000x reference)
"""Grouped-experts MoE MLP (Aria) on 8 TRN2 NeuronCores.

Expert parallelism: 8 experts / 8 cores -> each core owns one expert's
weights (w1 [2048, 8192], w2 [4096, 2048]) and processes that expert's
token block (tokens are pre-sorted by expert, so routing is host-side
slicing). No device collectives needed.

Per-core device kernel (all matmul compute in bf16, fp32 accumulate):
  phase 1: fc1T[icol, tok] = w1.T @ x.T   (icol on partitions)
           computed as 64 psum tiles [128, 128], j-loop pairs proj/gate
           hT[j] = silu(projT[j]) * gateT[j]   (fused at psum drain)
  phase 2: out[tok, hcol] = hT.T @ w2     (tokens on partitions)
           4 psum tiles [128, 512] accumulated over 32 inter k-tiles

Host pre-arranges weight layouts so every DMA is a plain contiguous
2D block, and casts to bf16 (halves the HBM traffic; the memory-bound
roofline is weight streaming).
"""

import sys
import types

sys.path.insert(0, "/opt/trn_rl_repo")

# This axon deployment ships without antenv.axon_hooks; shim it so
# bass_utils' trace path degrades gracefully instead of ImportError-ing.
try:
    import antenv  # noqa: F401

    if "antenv.axon_hooks" not in sys.modules:
        _hooks = types.ModuleType("antenv.axon_hooks")
        _hooks.get_axon_ntff_profile_hook = lambda: None
        sys.modules["antenv.axon_hooks"] = _hooks
except ImportError:
    pass

from contextlib import ExitStack

import ml_dtypes
import numpy as np

import concourse.bass as bass  # noqa: F401  (import keeps bass registered)
import concourse.tile as tile
from concourse import bacc, mybir
from concourse.bass_utils import run_bass_kernel_spmd

NUM_TOKENS = 1024
HIDDEN = 2048
INTER = 4096
EXPERTS = 8
N_CORES = 8
P = 128
T = 128  # tokens per core (padded)
KT1 = HIDDEN // P  # 16 k-tiles for matmul 1
JT = INTER // P  # 32 icol tiles per half (proj/gate)
NT2 = HIDDEN // 512  # 4 output column tiles of 512

BF16 = mybir.dt.bfloat16
F32 = mybir.dt.float32

_CACHE = {}


def _build():
    nc = bacc.Bacc(
        "TRN2", target_bir_lowering=False, debug=False, num_devices=N_CORES
    )
    xt_d = nc.dram_tensor("xt", [P, KT1 * T], BF16, kind="ExternalInput").ap()
    w1_d = nc.dram_tensor(
        "w1", [JT, 2, P, KT1 * P], BF16, kind="ExternalInput"
    ).ap()
    w2_d = nc.dram_tensor("w2", [JT, P, HIDDEN], BF16, kind="ExternalInput").ap()
    out_d = nc.dram_tensor("out", [T, HIDDEN], F32, kind="ExternalOutput").ap()

    with tile.TileContext(nc) as tc:
        with ExitStack() as ctx:
            xpool = ctx.enter_context(tc.tile_pool(name="x", bufs=1))
            w1pool = ctx.enter_context(tc.tile_pool(name="w1", bufs=6))
            w2pool = ctx.enter_context(tc.tile_pool(name="w2", bufs=4))
            hpool = ctx.enter_context(tc.tile_pool(name="h", bufs=1))
            spool = ctx.enter_context(tc.tile_pool(name="s", bufs=4))
            opool = ctx.enter_context(tc.tile_pool(name="o", bufs=1))
            psum1 = ctx.enter_context(tc.tile_pool(name="ps1", bufs=4, space="PSUM"))
            psum2 = ctx.enter_context(tc.tile_pool(name="ps2", bufs=1, space="PSUM"))

            xt = xpool.tile([P, KT1 * T], BF16)
            nc.sync.dma_start(xt[:], xt_d[:, :])

            hT = hpool.tile([P, JT * T], BF16)

            # phase 2 accumulators, held across the whole j loop
            po = [psum2.tile([P, 512], F32, name=f"po{n}") for n in range(NT2)]

            for j in range(JT):
                w1p = w1pool.tile([P, KT1 * P], BF16, tag="w1t")
                nc.sync.dma_start(w1p[:], w1_d[j, 0])
                w1g = w1pool.tile([P, KT1 * P], BF16, tag="w1t")
                nc.sync.dma_start(w1g[:], w1_d[j, 1])

                pa = psum1.tile([P, T], F32, tag="ps1t")
                pb = psum1.tile([P, T], F32, tag="ps1t")
                for k in range(KT1):
                    nc.tensor.matmul(
                        pa[:],
                        lhsT=w1p[:, k * P : (k + 1) * P],
                        rhs=xt[:, k * T : (k + 1) * T],
                        start=(k == 0),
                        stop=(k == KT1 - 1),
                    )
                for k in range(KT1):
                    nc.tensor.matmul(
                        pb[:],
                        lhsT=w1g[:, k * P : (k + 1) * P],
                        rhs=xt[:, k * T : (k + 1) * T],
                        start=(k == 0),
                        stop=(k == KT1 - 1),
                    )
                sa = spool.tile([P, T], F32, tag="silu")
                nc.scalar.activation(
                    sa[:], pa[:], mybir.ActivationFunctionType.Silu
                )
                nc.vector.tensor_mul(hT[:, j * T : (j + 1) * T], pb[:], sa[:])

            # phase 2
            for j in range(JT):
                w2t = w2pool.tile([P, HIDDEN], BF16, tag="w2t")
                nc.sync.dma_start(w2t[:], w2_d[j])
                for n in range(NT2):
                    nc.tensor.matmul(
                        po[n][:],
                        lhsT=hT[:, j * T : (j + 1) * T],
                        rhs=w2t[:, n * 512 : (n + 1) * 512],
                        start=(j == 0),
                        stop=(j == JT - 1),
                    )

            osb = opool.tile([T, HIDDEN], F32)
            for n in range(NT2):
                nc.scalar.copy(osb[:, n * 512 : (n + 1) * 512], po[n][:])
            nc.sync.dma_start(out_d[:, :], osb[:])

    nc.compile()
    return nc


def _get_nc():
    if "nc" not in _CACHE:
        _CACHE["nc"] = _build()
    return _CACHE["nc"]


def _prep_token_block(x_block: np.ndarray) -> np.ndarray:
    """[T, HIDDEN] f32 -> xt layout [P, KT1*T] bf16 where
    xt[p, k*T + t] = x_block[t, k*P + p]."""
    a = np.ascontiguousarray(
        x_block.T.reshape(KT1, P, T).transpose(1, 0, 2).reshape(P, KT1 * T)
    )
    return a.astype(ml_dtypes.bfloat16)


def _prep_w1(w1_e: np.ndarray) -> np.ndarray:
    """[HIDDEN, 2*INTER] f32 -> [JT, 2, P, KT1*P] bf16 where
    [j, pg, p, k*P + i] = w1_e[k*P + p, (pg*JT + j)*P + i]."""
    a = w1_e.reshape(KT1, P, 2 * JT, P).transpose(2, 1, 0, 3)  # [it, p, k, i]
    a = a.reshape(2, JT, P, KT1, P).transpose(1, 0, 2, 3, 4)  # [j, pg, p, k, i]
    return np.ascontiguousarray(a.reshape(JT, 2, P, KT1 * P)).astype(
        ml_dtypes.bfloat16
    )


def _prep_w2(w2_e: np.ndarray) -> np.ndarray:
    """[INTER, HIDDEN] f32 -> [JT, P, HIDDEN] bf16 where
    [j, p, c] = w2_e[j*P + p, c]."""
    return np.ascontiguousarray(w2_e.reshape(JT, P, HIDDEN)).astype(
        ml_dtypes.bfloat16
    )


def _run_device(in_maps):
    nc = _get_nc()
    res = run_bass_kernel_spmd(nc, in_maps, core_ids=list(range(N_CORES)))
    return [r["out"] for r in res.results]


def kernel(permuted_tokens, w1, w2, tokens_per_expert):
    permuted_tokens = np.asarray(permuted_tokens, dtype=np.float32)
    w1 = np.asarray(w1, dtype=np.float32)
    w2 = np.asarray(w2, dtype=np.float32)
    counts = np.asarray(tokens_per_expert).astype(np.int64)

    n = permuted_tokens.shape[0]
    bounds = np.minimum(np.cumsum(counts), n)
    starts = np.concatenate([[0], bounds[:-1]])
    eff_counts = np.maximum(bounds - starts, 0)

    w1_maps = [_prep_w1(w1[e]) for e in range(EXPERTS)]
    w2_maps = [_prep_w2(w2[e]) for e in range(EXPERTS)]

    out = np.zeros((n, HIDDEN), dtype=np.float32)
    rounds = int(max(1, -(-int(eff_counts.max()) // T)))
    for r in range(rounds):
        in_maps = []
        chunk_info = []
        for e in range(EXPERTS):
            c0 = starts[e] + r * T
            cnt = int(min(max(eff_counts[e] - r * T, 0), T))
            blk = np.zeros((T, HIDDEN), dtype=np.float32)
            if cnt > 0:
                blk[:cnt] = permuted_tokens[c0 : c0 + cnt]
            chunk_info.append((c0, cnt))
            in_maps.append(
                {"xt": _prep_token_block(blk), "w1": w1_maps[e], "w2": w2_maps[e]}
            )
        outs = _run_device(in_maps)
        for e in range(EXPERTS):
            c0, cnt = chunk_info[e]
            if cnt > 0:
                out[c0 : c0 + cnt] = outs[e][:cnt]
    return out
